# revision 1
# baseline (speedup 1.0000x reference)
"""Trainium2 Bass kernel for CrossAttention (B=4, N=2048, C=768, H=12).

Sharding: 8 cores = 4 head-groups (3 heads each) x 2 batch-groups (2 batches
each). Every core computes, for its (heads, batches):
    Q/K/V projections -> S^T = K @ Q^T + bias^T -> exp -> PV (ones-augmented V
    gives softmax sums for free) -> normalize -> partial output projection.
Host pre-transposes inputs to [.., C|*, N] layouts (so no on-chip transposes
are ever needed) and converts to bf16; host sums the 4 head-group partial
outputs at the end and adds the projection bias.
"""

import sys

for _p in ("/opt/trn_rl_repo",):
    if _p not in sys.path:
        sys.path.insert(0, _p)

import numpy as np
import ml_dtypes

B, N, C, H, D = 4, 2048, 768, 12, 64
SCALE = D ** -0.5
HG, BG = 4, 2            # head-groups x batch-groups = 8 cores
HL = H // HG             # 3 heads per core
BL = B // BG             # 2 batches per core
NB = 4                   # n blocks per row strip
NBS = N // NB            # 512 (= one PSUM bank of f32)
MT = N // 128            # 16 m tiles
CT = C // 128            # 6 c tiles
BF16 = ml_dtypes.bfloat16

# fraction of (mt, nb) bias-add work done on the tensor engine (PSUM inject)
# instead of the vector engine; tuned from profiles.
PE_BIAS_EVERY = 0        # 0 = disabled (all bias adds on DVE)

_prog_cache = {}


def _build_program(debug=False):
    import concourse.bass as bass
    import concourse.tile as tile
    from concourse import bacc, mybir
    from concourse.tile_rust import add_dep_helper

    f32 = mybir.dt.float32
    bf16 = mybir.dt.bfloat16

    nc = bacc.Bacc("TRN2", target_bir_lowering=False, debug=False)

    xT = nc.dram_tensor("xT", [BL, C, N], bf16, kind="ExternalInput")
    kT = nc.dram_tensor("kT", [BL, C, N], bf16, kind="ExternalInput")
    vT = nc.dram_tensor("vT", [BL, C, N], bf16, kind="ExternalInput")
    bT = nc.dram_tensor("bT", [HL, N, N], bf16, kind="ExternalInput")  # [h, m, n]
    wq = nc.dram_tensor("wq", [C, HL * D], bf16, kind="ExternalInput")
    wk = nc.dram_tensor("wk", [C, HL * D], bf16, kind="ExternalInput")
    wv = nc.dram_tensor("wv", [C, HL * D], bf16, kind="ExternalInput")
    wp = nc.dram_tensor("wp", [HL * D, C], bf16, kind="ExternalInput")
    ones = nc.dram_tensor("ones", [1, D], bf16, kind="ExternalInput")
    ident = nc.dram_tensor("ident", [128, 128], bf16, kind="ExternalInput")
    yT = nc.dram_tensor("yT", [BL, C, N], f32, kind="ExternalOutput")
    if debug:
        dbg_q = nc.dram_tensor("dbg_q", [128, N], bf16, kind="ExternalOutput")
        dbg_k = nc.dram_tensor("dbg_k", [128, N], bf16, kind="ExternalOutput")
        dbg_v = nc.dram_tensor("dbg_v", [128, MT * (D + 1)], bf16,
                               kind="ExternalOutput")
        dbg_on = nc.dram_tensor("dbg_on", [128, N], bf16, kind="ExternalOutput")
        dbg_sum = nc.dram_tensor("dbg_sum", [1, N], f32, kind="ExternalOutput")
        dbg_rec = nc.dram_tensor("dbg_rec", [1, N], f32, kind="ExternalOutput")

    GD = HL * D  # 192

    def mm(out_ap, lhsT, rhs, start, stop, **kw):
        assert rhs.shape[-1] <= 512
        nc.tensor.matmul(out_ap[:], lhsT, rhs, start=start, stop=stop, **kw)

    with tile.TileContext(nc) as tc:
        with (
            tc.tile_pool(name="wpool", bufs=1) as wpool,
            tc.tile_pool(name="stream", bufs=7) as stream,
            tc.tile_pool(name="persist", bufs=1) as persist,
            tc.tile_pool(name="biasp", bufs=4) as biasp,
            tc.tile_pool(name="ppool", bufs=5) as ppool,
            tc.tile_pool(name="miscp", bufs=2) as miscp,
            tc.tile_pool(name="ypool", bufs=3) as ypool,
            tc.tile_pool(name="ps", bufs=2, space="PSUM") as ps,
            tc.tile_pool(name="po", bufs=4, space="PSUM") as po,
        ):
            # ---- constants / weights ----
            wq_sb = wpool.tile([128, CT * GD], bf16, tag="wq")
            nc.sync.dma_start(wq_sb.rearrange("p (t d) -> p t d", d=GD),
                              wq.rearrange("(t p) d -> p t d", p=128))
            wk_sb = wpool.tile([128, CT * GD], bf16, tag="wk")
            nc.sync.dma_start(wk_sb.rearrange("p (t d) -> p t d", d=GD),
                              wk.rearrange("(t p) d -> p t d", p=128))
            wv_sb = wpool.tile([128, CT * GD], bf16, tag="wv")
            nc.sync.dma_start(wv_sb.rearrange("p (t d) -> p t d", d=GD),
                              wv.rearrange("(t p) d -> p t d", p=128))
            wp0_sb = wpool.tile([128, C], bf16, tag="wp0")
            nc.sync.dma_start(wp0_sb[:], wp[0:128, :])
            wp1_sb = wpool.tile([64, C], bf16, tag="wp1")
            nc.sync.dma_start(wp1_sb[:], wp[128:192, :])
            ones_sb = wpool.tile([1, D], bf16, tag="ones")
            nc.sync.dma_start(ones_sb[:], ones[:, :])
            id_sb = wpool.tile([128, 128], bf16, tag="ident")
            nc.sync.dma_start(id_sb[:], ident[:, :])

            # head groups: (psum/Q/K partition offset, size); heads 0,1 packed
            groups = [(0, 128), (128, 64)]

            # ---- persistent per-batch tensors ----
            qT01, qT2, kT01, kT2 = {}, {}, {}, {}
            vaug = {}
            on01, on2 = {}, {}
            for b in range(BL):
                qT01[b] = persist.tile([128, N], bf16, tag=f"q01_{b}", name=f"q01_{b}")
                qT2[b] = persist.tile([64, N], bf16, tag=f"q2_{b}", name=f"q2_{b}")
                kT01[b] = persist.tile([128, N], bf16, tag=f"k01_{b}", name=f"k01_{b}")
                kT2[b] = persist.tile([64, N], bf16, tag=f"k2_{b}", name=f"k2_{b}")
                on01[b] = persist.tile([128, N], bf16, tag=f"on01_{b}", name=f"on01_{b}")
                on2[b] = persist.tile([64, N], bf16, tag=f"on2_{b}", name=f"on2_{b}")
                for h in range(HL):
                    vaug[(b, h)] = persist.tile([128, MT * (D + 1)], bf16,
                                                tag=f"v_{b}_{h}", name=f"v_{b}_{h}")
                    # ones column for softmax-sum augmentation
                    va3 = vaug[(b, h)].rearrange("p (t c) -> p t c", c=D + 1)
                    nc.gpsimd.memset(va3[:, :, D], 1.0)

            # =========== phase 1: projections ===========
            for b in range(BL):
                # Q
                xt = []
                for ct in range(CT):
                    t = stream.tile([128, N], bf16, tag="stream", name="stream_t")
                    nc.gpsimd.dma_start(t[:], xT[b, ct * 128:(ct + 1) * 128, :])
                    xt.append(t)
                for goff, gsz in groups:
                    for nb in range(NB):
                        pq = ps.tile([gsz, NBS], f32, tag="s", name="ps_s")
                        for ct in range(CT):
                            mm(pq,
                               wq_sb[:, ct * GD + goff: ct * GD + goff + gsz],
                               xt[ct][:, nb * NBS:(nb + 1) * NBS],
                               start=(ct == 0), stop=(ct == CT - 1))
                        dst = qT01[b] if gsz == 128 else qT2[b]
                        nc.vector.tensor_copy(
                            dst[:, nb * NBS:(nb + 1) * NBS], pq[:])
                # K
                kt = []
                for ct in range(CT):
                    t = stream.tile([128, N], bf16, tag="stream", name="stream_t")
                    nc.gpsimd.dma_start(t[:], kT[b, ct * 128:(ct + 1) * 128, :])
                    kt.append(t)
                for goff, gsz in groups:
                    for nb in range(NB):
                        pk = ps.tile([gsz, NBS], f32, tag="s", name="ps_s")
                        for ct in range(CT):
                            mm(pk,
                               wk_sb[:, ct * GD + goff: ct * GD + goff + gsz],
                               kt[ct][:, nb * NBS:(nb + 1) * NBS],
                               start=(ct == 0), stop=(ct == CT - 1))
                        dst = kT01[b] if gsz == 128 else kT2[b]
                        nc.vector.tensor_copy(
                            dst[:, nb * NBS:(nb + 1) * NBS], pk[:])
                # V (layout [m, d] with ones column at d=64 per m-tile)
                vt = []
                for ct in range(CT):
                    t = stream.tile([128, N], bf16, tag="stream", name="stream_t")
                    nc.gpsimd.dma_start(t[:], vT[b, ct * 128:(ct + 1) * 128, :])
                    vt.append(t)
                for mt in range(MT):
                    pv = ps.tile([128, GD], f32, tag="s", name="ps_v")
                    for ct in range(CT):
                        nc.tensor.matmul(
                            pv[:],
                            vt[ct][:, mt * 128:(mt + 1) * 128],
                            wv_sb[:, ct * GD:(ct + 1) * GD],
                            start=(ct == 0), stop=(ct == CT - 1),
                        )
                    for h in range(HL):
                        nc.vector.tensor_copy(
                            vaug[(b, h)][:, mt * (D + 1): mt * (D + 1) + D],
                            pv[:, h * D:(h + 1) * D])

            # =========== phase 2: attention per (b, h) ===========
            for b in range(BL):
                for h in range(HL):
                    if h < 2:
                        k_src = kT01[b][h * D:(h + 1) * D, :]
                        q_src = qT01[b][h * D:(h + 1) * D, :]
                    else:
                        k_src = kT2[b][:, :]
                        q_src = qT2[b][:, :]

                    pos = [po.tile([D + 1, 512], f32, tag="o", name="po_o")
                           for _ in range(4)]
                    pts = {}
                    # software pipeline: PV trails QK by one m-tile so the
                    # tensor engine never waits on the exp
                    for mt in range(MT + 1):
                        if mt < MT:
                            bt = biasp.tile([128, N], bf16, tag="bias",
                                            name="bias_t")
                            nc.sync.dma_start(
                                bt[:], bT[h, mt * 128:(mt + 1) * 128, :])
                            for nb2 in range(2):
                                sp = ps.tile([128, 1024], f32, tag="s",
                                             name="ps_sc")
                                injs = []
                                for hf in range(2):
                                    off = nb2 * 1024 + hf * 512
                                    # scores then bias, accumulated in PSUM
                                    qk_i = nc.tensor.matmul(
                                        sp[:, hf * 512:(hf + 1) * 512],
                                        k_src[:, mt * 128:(mt + 1) * 128],
                                        q_src[:, off:off + 512],
                                        start=True, stop=False)
                                    inj_i = nc.tensor.matmul(
                                        sp[:, hf * 512:(hf + 1) * 512],
                                        id_sb[:], bt[:, off:off + 512],
                                        start=False, stop=True)
                                    add_dep_helper(inj_i.ins, qk_i.ins,
                                                   reason="bias after scores")
                                    injs.append(inj_i)
                                pt = ppool.tile([128, 1024], bf16, tag="p",
                                                name="p_t")
                                exp_i = nc.scalar.activation(
                                    pt[:], sp[:],
                                    mybir.ActivationFunctionType.Exp)
                                for inj_i in injs:
                                    add_dep_helper(exp_i.ins, inj_i.ins,
                                                   reason="exp after bias")
                                pts[(mt, nb2)] = pt
                        if mt > 0:
                            pm = mt - 1
                            vsl = vaug[(b, h)][:,
                                               pm * (D + 1):(pm + 1) * (D + 1)]
                            for nb2 in range(2):
                                pt = pts.pop((pm, nb2))
                                for hf in range(2):
                                    nc.tensor.matmul(
                                        pos[nb2 * 2 + hf][:], vsl,
                                        pt[:, hf * 512:(hf + 1) * 512],
                                        start=(pm == 0), stop=(pm == MT - 1))

                    # normalization: rows 0..63 of pos are O^T, row 64 = sums
                    sum_sb = miscp.tile([1, N], f32, tag="sum_sb", name="sum_sb")
                    rec_f = miscp.tile([1, N], f32, tag="rec_f", name="rec_f")
                    rec_b = miscp.tile([1, N], bf16, tag="rec_b", name="rec_b")
                    for q4 in range(4):
                        # custom-DVE recip can't read PSUM; stage sums in SBUF
                        nc.vector.tensor_copy(
                            sum_sb[:, q4 * 512:(q4 + 1) * 512],
                            pos[q4][D:D + 1, :])
                    nc.vector.reciprocal_approx_fast(rec_f[:], sum_sb[:])
                    nc.scalar.copy(rec_b[:], rec_f[:])
                    if debug and b == 0 and h == 0:
                        nc.sync.dma_start(dbg_sum[:, :], sum_sb[:])
                        nc.sync.dma_start(dbg_rec[:, :], rec_f[:])
                    for nb2 in range(2):
                        r_ps = ps.tile([D, 1024], f32, tag="s", name="ps_r")
                        for hf in range(2):
                            nc.tensor.matmul(
                                r_ps[:, hf * 512:(hf + 1) * 512], ones_sb[:],
                                rec_b[:, nb2 * 1024 + hf * 512:
                                      nb2 * 1024 + (hf + 1) * 512],
                                start=True, stop=True)
                        r_sb = miscp.tile([D, 1024], bf16, tag="r_sb",
                                          name="r_sb")
                        nc.vector.tensor_copy(r_sb[:], r_ps[:])
                        if h < 2:
                            dst0 = on01[b][h * D:(h + 1) * D,
                                           nb2 * 1024:(nb2 + 1) * 1024]
                        else:
                            dst0 = on2[b][:, nb2 * 1024:(nb2 + 1) * 1024]
                        for hf in range(2):
                            nc.vector.tensor_mul(
                                dst0[:, hf * 512:(hf + 1) * 512],
                                pos[nb2 * 2 + hf][0:D, :], r_sb[:, hf * 512:(hf + 1) * 512])
                # ---- output projection for batch b (after all its heads) ----
                if h == HL - 1:
                    if debug and b == 0:
                        nc.sync.dma_start(dbg_q[:, :], qT01[0][:])
                        nc.sync.dma_start(dbg_k[:, :], kT01[0][:])
                        nc.sync.dma_start(dbg_v[:, :], vaug[(0, 0)][:])
                        nc.sync.dma_start(dbg_on[:, :], on01[0][:])
                    for ct in range(CT):
                        y_sb = ypool.tile([128, N], f32, tag="y", name="y_t")
                        for nb2 in range(2):
                            py = ps.tile([128, 1024], f32, tag="s",
                                         name="ps_sc")
                            for hf in range(2):
                                sl = slice(nb2 * 1024 + hf * 512,
                                           nb2 * 1024 + (hf + 1) * 512)
                                nc.tensor.matmul(
                                    py[:, hf * 512:(hf + 1) * 512],
                                    wp0_sb[:, ct * 128:(ct + 1) * 128],
                                    on01[b][:, sl], start=True, stop=False)
                                nc.tensor.matmul(
                                    py[:, hf * 512:(hf + 1) * 512],
                                    wp1_sb[:, ct * 128:(ct + 1) * 128],
                                    on2[b][:, sl], start=False, stop=True)
                            nc.scalar.copy(
                                y_sb[:, nb2 * 1024:(nb2 + 1) * 1024], py[:])
                        nc.gpsimd.dma_start(
                            yT[b, ct * 128:(ct + 1) * 128, :], y_sb[:])
    nc.compile()
    return nc


def get_program(debug=False):
    key = ("nc", debug)
    if key not in _prog_cache:
        _prog_cache[key] = _build_program(debug)
    return _prog_cache[key]


def make_in_maps(x, k_in, v_in, rel_pos_bias, Wq, Wk, Wv, Wp):
    xT = x.transpose(0, 2, 1).astype(BF16)
    kT = k_in.transpose(0, 2, 1).astype(BF16)
    vT = v_in.transpose(0, 2, 1).astype(BF16)
    bT = rel_pos_bias.transpose(0, 2, 1).astype(BF16)       # [H, m, n]
    WqT = (Wq * SCALE).T.astype(BF16)                       # [C, C]
    WkT = Wk.T.astype(BF16)
    WvT = Wv.T.astype(BF16)
    WpT = Wp.T.astype(BF16)                                 # [C(d_in), C]
    ones = np.ones((1, D), dtype=BF16)
    ident = np.eye(128, dtype=BF16)

    in_maps = []
    for c in range(8):
        hg, bg = c % HG, c // HG
        hs, bs = hg * HL, bg * BL
        in_maps.append({
            "xT": np.ascontiguousarray(xT[bs:bs + BL]),
            "kT": np.ascontiguousarray(kT[bs:bs + BL]),
            "vT": np.ascontiguousarray(vT[bs:bs + BL]),
            "bT": np.ascontiguousarray(bT[hs:hs + HL]),
            "wq": np.ascontiguousarray(WqT[:, hs * D:(hs + HL) * D]),
            "wk": np.ascontiguousarray(WkT[:, hs * D:(hs + HL) * D]),
            "wv": np.ascontiguousarray(WvT[:, hs * D:(hs + HL) * D]),
            "wp": np.ascontiguousarray(WpT[hs * D:(hs + HL) * D, :]),
            "ones": ones,
            "ident": ident,
        })
    return in_maps


def assemble_output(results, bp):
    y = np.zeros((B, C, N), dtype=np.float32)
    for c in range(8):
        hg, bg = c % HG, c // HG
        bs = bg * BL
        y[bs:bs + BL] += results[c]["yT"]
    out = y.transpose(0, 2, 1) + bp.astype(np.float32)
    return np.ascontiguousarray(out.astype(np.float32))


def kernel(**inputs):
    from concourse.bass_utils import run_bass_kernel_spmd

    x = np.asarray(inputs["x"], dtype=np.float32)
    k_in = np.asarray(inputs["k_in"], dtype=np.float32)
    v_in = np.asarray(inputs["v_in"], dtype=np.float32)
    rel_pos_bias = np.asarray(inputs["rel_pos_bias"], dtype=np.float32)
    Wq = np.asarray(inputs["Wq"], dtype=np.float32)
    Wk = np.asarray(inputs["Wk"], dtype=np.float32)
    Wv = np.asarray(inputs["Wv"], dtype=np.float32)
    Wp = np.asarray(inputs["Wp"], dtype=np.float32)
    bp = np.asarray(inputs["bp"], dtype=np.float32)

    nc = get_program()
    in_maps = make_in_maps(x, k_in, v_in, rel_pos_bias, Wq, Wk, Wv, Wp)
    res = run_bass_kernel_spmd(nc, in_maps, list(range(8)))
    return assemble_output(res.results, bp)



# revision 2
# speedup vs baseline: 1.3361x; 1.3361x over previous
"""Trainium2 Bass kernel for CrossAttention (B=4, N=2048, C=768, H=12).

Sharding: 8 cores = 4 head-groups (3 heads each) x 2 batch-groups (2 batches
each). Every core computes, for its (heads, batches):
    Q/K/V projections -> S^T = K @ Q^T -> exp(S^T) * exp(bias)^T (host
    precomputes EB = exp(bias); the multiply runs on the DVE in 4x bf16 mode,
    keeping the bias add off the tensor engine entirely) -> PV (ones-augmented
    V gives softmax sums for free) -> normalize -> partial output projection.
Loop order h -> b -> m-tile keeps each head's 16 EB tiles resident in SBUF so
they are DMA'd once per head instead of once per (head, batch).
Host pre-transposes inputs to [.., C|*, N] layouts (so no on-chip transposes
are ever needed) and converts to bf16; host sums the 4 head-group partial
outputs (bf16) at the end and adds the projection bias.
"""

import sys

for _p in ("/opt/trn_rl_repo",):
    if _p not in sys.path:
        sys.path.insert(0, _p)

import numpy as np
import ml_dtypes

B, N, C, H, D = 4, 2048, 768, 12, 64
SCALE = D ** -0.5
HG, BG = 4, 2            # head-groups x batch-groups = 8 cores
HL = H // HG             # 3 heads per core
BL = B // BG             # 2 batches per core
NB = 4                   # n blocks per row strip
NBS = N // NB            # 512 (= one PSUM bank of f32)
MT = N // 128            # 16 m tiles
CT = C // 128            # 6 c tiles
BF16 = ml_dtypes.bfloat16

_prog_cache = {}


def _build_program():
    import concourse.bass as bass
    import concourse.tile as tile
    from concourse import bacc, mybir

    f32 = mybir.dt.float32
    bf16 = mybir.dt.bfloat16

    nc = bacc.Bacc("TRN2", target_bir_lowering=False, debug=False)

    xT = nc.dram_tensor("xT", [BL, C, N], bf16, kind="ExternalInput")
    kT = nc.dram_tensor("kT", [BL, C, N], bf16, kind="ExternalInput")
    vT = nc.dram_tensor("vT", [BL, C, N], bf16, kind="ExternalInput")
    ebT = nc.dram_tensor("ebT", [HL, N, N], bf16, kind="ExternalInput")  # exp(bias)^T [h, m, n]
    wq = nc.dram_tensor("wq", [C, HL * D], bf16, kind="ExternalInput")
    wk = nc.dram_tensor("wk", [C, HL * D], bf16, kind="ExternalInput")
    wv = nc.dram_tensor("wv", [C, HL * D], bf16, kind="ExternalInput")
    wp = nc.dram_tensor("wp", [HL * D, C], bf16, kind="ExternalInput")
    ones = nc.dram_tensor("ones", [1, D], bf16, kind="ExternalInput")
    yT = nc.dram_tensor("yT", [BL, C, N], bf16, kind="ExternalOutput")

    GD = HL * D  # 192

    with tile.TileContext(nc) as tc:
        with (
            tc.tile_pool(name="wpool", bufs=1) as wpool,
            tc.tile_pool(name="bigp", bufs=16) as bigp,
            tc.tile_pool(name="persist", bufs=1) as persist,
            tc.tile_pool(name="ppool", bufs=1) as ppool,
            tc.tile_pool(name="miscp", bufs=1) as miscp,
            tc.tile_pool(name="ypool", bufs=2) as ypool,
            tc.tile_pool(name="ps", bufs=2, space="PSUM") as ps,
            tc.tile_pool(name="po", bufs=4, space="PSUM") as po,
        ):
            # ---- constants / weights ----
            wq_sb = wpool.tile([128, CT * GD], bf16, tag="wq")
            nc.sync.dma_start(wq_sb.rearrange("p (t d) -> p t d", d=GD),
                              wq.rearrange("(t p) d -> p t d", p=128))
            wk_sb = wpool.tile([128, CT * GD], bf16, tag="wk")
            nc.sync.dma_start(wk_sb.rearrange("p (t d) -> p t d", d=GD),
                              wk.rearrange("(t p) d -> p t d", p=128))
            wv_sb = wpool.tile([128, CT * GD], bf16, tag="wv")
            nc.sync.dma_start(wv_sb.rearrange("p (t d) -> p t d", d=GD),
                              wv.rearrange("(t p) d -> p t d", p=128))
            wp0_sb = wpool.tile([128, C], bf16, tag="wp0")
            nc.sync.dma_start(wp0_sb[:], wp[0:128, :])
            wp1_sb = wpool.tile([64, C], bf16, tag="wp1")
            nc.sync.dma_start(wp1_sb[:], wp[128:192, :])
            ones_sb = wpool.tile([1, D], bf16, tag="ones")
            nc.sync.dma_start(ones_sb[:], ones[:, :])

            # head groups: (psum/Q/K partition offset, size); heads 0,1 packed
            groups = [(0, 128), (128, 64)]

            # ---- persistent per-batch tensors ----
            qT01, qT2, kT01, kT2 = {}, {}, {}, {}
            vaug = {}
            on01, on2 = {}, {}
            for b in range(BL):
                qT01[b] = persist.tile([128, N], bf16, tag=f"q01_{b}", name=f"q01_{b}")
                qT2[b] = persist.tile([64, N], bf16, tag=f"q2_{b}", name=f"q2_{b}")
                kT01[b] = persist.tile([128, N], bf16, tag=f"k01_{b}", name=f"k01_{b}")
                kT2[b] = persist.tile([64, N], bf16, tag=f"k2_{b}", name=f"k2_{b}")
                on01[b] = persist.tile([128, N], bf16, tag=f"on01_{b}", name=f"on01_{b}")
                on2[b] = persist.tile([64, N], bf16, tag=f"on2_{b}", name=f"on2_{b}")
                for h in range(HL):
                    vaug[(b, h)] = persist.tile([128, MT * (D + 1)], bf16,
                                                tag=f"v_{b}_{h}", name=f"v_{b}_{h}")
                    # ones column for softmax-sum augmentation
                    va3 = vaug[(b, h)].rearrange("p (t c) -> p t c", c=D + 1)
                    nc.gpsimd.memset(va3[:, :, D], 1.0)

            # =========== phase 1: projections ===========
            for b in range(BL):
                # Q
                xt = []
                for ct in range(CT):
                    t = bigp.tile([128, N], bf16, tag="big", name="big_t")
                    nc.gpsimd.dma_start(t[:], xT[b, ct * 128:(ct + 1) * 128, :])
                    xt.append(t)
                for goff, gsz in groups:
                    for nb in range(NB):
                        pq = ps.tile([gsz, NBS], f32, tag="s", name="ps_s")
                        for ct in range(CT):
                            nc.tensor.matmul(
                                pq[:],
                                wq_sb[:, ct * GD + goff: ct * GD + goff + gsz],
                                xt[ct][:, nb * NBS:(nb + 1) * NBS],
                                start=(ct == 0), stop=(ct == CT - 1))
                        dst = qT01[b] if gsz == 128 else qT2[b]
                        nc.vector.tensor_copy(
                            dst[:, nb * NBS:(nb + 1) * NBS], pq[:])
                # K
                kt = []
                for ct in range(CT):
                    t = bigp.tile([128, N], bf16, tag="big", name="big_t")
                    nc.gpsimd.dma_start(t[:], kT[b, ct * 128:(ct + 1) * 128, :])
                    kt.append(t)
                for goff, gsz in groups:
                    for nb in range(NB):
                        pk = ps.tile([gsz, NBS], f32, tag="s", name="ps_s")
                        for ct in range(CT):
                            nc.tensor.matmul(
                                pk[:],
                                wk_sb[:, ct * GD + goff: ct * GD + goff + gsz],
                                kt[ct][:, nb * NBS:(nb + 1) * NBS],
                                start=(ct == 0), stop=(ct == CT - 1))
                        dst = kT01[b] if gsz == 128 else kT2[b]
                        nc.vector.tensor_copy(
                            dst[:, nb * NBS:(nb + 1) * NBS], pk[:])
                # V (layout [m, d] with ones column at d=64 per m-tile)
                vt = []
                for ct in range(CT):
                    t = bigp.tile([128, N], bf16, tag="big", name="big_t")
                    nc.gpsimd.dma_start(t[:], vT[b, ct * 128:(ct + 1) * 128, :])
                    vt.append(t)
                for mt in range(MT):
                    pv = ps.tile([128, GD], f32, tag="s", name="ps_v")
                    for ct in range(CT):
                        nc.tensor.matmul(
                            pv[:],
                            vt[ct][:, mt * 128:(mt + 1) * 128],
                            wv_sb[:, ct * GD:(ct + 1) * GD],
                            start=(ct == 0), stop=(ct == CT - 1),
                        )
                    for h in range(HL):
                        nc.vector.tensor_copy(
                            vaug[(b, h)][:, mt * (D + 1): mt * (D + 1) + D],
                            pv[:, h * D:(h + 1) * D])

            # =========== phase 2: attention, h outer so EB loads once ===========
            PF = 2  # EB prefetch depth (m-tiles ahead)
            for h in range(HL):
                eb = {}

                def load_eb(mt):
                    t = bigp.tile([128, N], bf16, tag="big", name="eb_t")
                    nc.sync.dma_start(t[:], ebT[h, mt * 128:(mt + 1) * 128, :])
                    eb[mt] = t

                for b in range(BL):
                    if h < 2:
                        k_src = kT01[b][h * D:(h + 1) * D, :]
                        q_src = qT01[b][h * D:(h + 1) * D, :]
                    else:
                        k_src = kT2[b][:, :]
                        q_src = qT2[b][:, :]

                    if b == 0:
                        for mt in range(PF):
                            load_eb(mt)

                    pos = [po.tile([D + 1, 512], f32, tag="o", name="po_o")
                           for _ in range(4)]
                    pts = {}
                    # software pipeline: PV trails QK by one m-tile so the
                    # tensor engine never waits on the exp
                    for mt in range(MT + 1):
                        if mt < MT:
                            if b == 0 and mt + PF < MT:
                                load_eb(mt + PF)
                            ebt = eb[mt]
                            for nb2 in range(2):
                                sp = ps.tile([128, 1024], f32, tag="s",
                                             name="ps_sc")
                                for hf in range(2):
                                    off = nb2 * 1024 + hf * 512
                                    nc.tensor.matmul(
                                        sp[:, hf * 512:(hf + 1) * 512],
                                        k_src[:, mt * 128:(mt + 1) * 128],
                                        q_src[:, off:off + 512],
                                        start=True, stop=True)
                                pt = ppool.tile([128, 1024], bf16, tag="pe",
                                                bufs=3, name="p_e")
                                nc.scalar.activation(
                                    pt[:], sp[:],
                                    mybir.ActivationFunctionType.Exp)
                                pt2 = ppool.tile([128, 1024], bf16, tag="pm",
                                                 bufs=5, name="p_m")
                                nc.vector.tensor_mul(
                                    pt2[:], pt[:],
                                    ebt[:, nb2 * 1024:(nb2 + 1) * 1024])
                                pts[(mt, nb2)] = pt2
                        if mt > 0:
                            pm = mt - 1
                            vsl = vaug[(b, h)][:,
                                               pm * (D + 1):(pm + 1) * (D + 1)]
                            for nb2 in range(2):
                                pt2 = pts.pop((pm, nb2))
                                for hf in range(2):
                                    nc.tensor.matmul(
                                        pos[nb2 * 2 + hf][:], vsl,
                                        pt2[:, hf * 512:(hf + 1) * 512],
                                        start=(pm == 0), stop=(pm == MT - 1))

                    # normalization: rows 0..63 of pos are O^T, row 64 = sums
                    sum_sb = miscp.tile([1, N], f32, tag="sum_sb", name="sum_sb")
                    rec_f = miscp.tile([1, N], f32, tag="rec_f", name="rec_f")
                    rec_b = miscp.tile([1, N], bf16, tag="rec_b", name="rec_b")
                    for q4 in range(4):
                        # custom-DVE recip can't read PSUM; stage sums in SBUF
                        nc.vector.tensor_copy(
                            sum_sb[:, q4 * 512:(q4 + 1) * 512],
                            pos[q4][D:D + 1, :])
                    nc.vector.reciprocal_approx_fast(rec_f[:], sum_sb[:])
                    nc.scalar.copy(rec_b[:], rec_f[:])
                    for nb2 in range(2):
                        r_ps = ps.tile([D, 1024], f32, tag="s", name="ps_r")
                        for hf in range(2):
                            nc.tensor.matmul(
                                r_ps[:, hf * 512:(hf + 1) * 512], ones_sb[:],
                                rec_b[:, nb2 * 1024 + hf * 512:
                                      nb2 * 1024 + (hf + 1) * 512],
                                start=True, stop=True)
                        r_sb = miscp.tile([D, 1024], bf16, tag="r_sb",
                                          bufs=2, name="r_sb")
                        nc.vector.tensor_copy(r_sb[:], r_ps[:])
                        if h < 2:
                            dst0 = on01[b][h * D:(h + 1) * D,
                                           nb2 * 1024:(nb2 + 1) * 1024]
                        else:
                            dst0 = on2[b][:, nb2 * 1024:(nb2 + 1) * 1024]
                        for hf in range(2):
                            nc.vector.tensor_mul(
                                dst0[:, hf * 512:(hf + 1) * 512],
                                pos[nb2 * 2 + hf][0:D, :],
                                r_sb[:, hf * 512:(hf + 1) * 512])

            # =========== phase 3: output projection ===========
            for b in range(BL):
                for ct in range(CT):
                    y_sb = ypool.tile([128, N], bf16, tag="y", name="y_t")
                    for nb2 in range(2):
                        py = ps.tile([128, 1024], f32, tag="s", name="ps_sc")
                        for hf in range(2):
                            sl = slice(nb2 * 1024 + hf * 512,
                                       nb2 * 1024 + (hf + 1) * 512)
                            nc.tensor.matmul(
                                py[:, hf * 512:(hf + 1) * 512],
                                wp0_sb[:, ct * 128:(ct + 1) * 128],
                                on01[b][:, sl], start=True, stop=False)
                            nc.tensor.matmul(
                                py[:, hf * 512:(hf + 1) * 512],
                                wp1_sb[:, ct * 128:(ct + 1) * 128],
                                on2[b][:, sl], start=False, stop=True)
                        nc.scalar.copy(
                            y_sb[:, nb2 * 1024:(nb2 + 1) * 1024], py[:])
                    nc.gpsimd.dma_start(
                        yT[b, ct * 128:(ct + 1) * 128, :], y_sb[:])
    nc.compile()
    return nc


def get_program():
    key = "nc"
    if key not in _prog_cache:
        _prog_cache[key] = _build_program()
    return _prog_cache[key]


def make_in_maps(x, k_in, v_in, rel_pos_bias, Wq, Wk, Wv, Wp):
    xT = x.transpose(0, 2, 1).astype(BF16)
    kT = k_in.transpose(0, 2, 1).astype(BF16)
    vT = v_in.transpose(0, 2, 1).astype(BF16)
    ebT = np.exp(rel_pos_bias.transpose(0, 2, 1)).astype(BF16)  # [H, m, n]
    WqT = (Wq * SCALE).T.astype(BF16)                       # [C, C]
    WkT = Wk.T.astype(BF16)
    WvT = Wv.T.astype(BF16)
    WpT = Wp.T.astype(BF16)                                 # [C(d_in), C]
    ones = np.ones((1, D), dtype=BF16)

    in_maps = []
    for c in range(8):
        hg, bg = c % HG, c // HG
        hs, bs = hg * HL, bg * BL
        in_maps.append({
            "xT": np.ascontiguousarray(xT[bs:bs + BL]),
            "kT": np.ascontiguousarray(kT[bs:bs + BL]),
            "vT": np.ascontiguousarray(vT[bs:bs + BL]),
            "ebT": np.ascontiguousarray(ebT[hs:hs + HL]),
            "wq": np.ascontiguousarray(WqT[:, hs * D:(hs + HL) * D]),
            "wk": np.ascontiguousarray(WkT[:, hs * D:(hs + HL) * D]),
            "wv": np.ascontiguousarray(WvT[:, hs * D:(hs + HL) * D]),
            "wp": np.ascontiguousarray(WpT[hs * D:(hs + HL) * D, :]),
            "ones": ones,
        })
    return in_maps


def assemble_output(results, bp):
    y = np.zeros((B, C, N), dtype=np.float32)
    for c in range(8):
        hg, bg = c % HG, c // HG
        bs = bg * BL
        y[bs:bs + BL] += results[c]["yT"].astype(np.float32)
    out = y.transpose(0, 2, 1) + bp.astype(np.float32)
    return np.ascontiguousarray(out.astype(np.float32))


def kernel(**inputs):
    from concourse.bass_utils import run_bass_kernel_spmd

    x = np.asarray(inputs["x"], dtype=np.float32)
    k_in = np.asarray(inputs["k_in"], dtype=np.float32)
    v_in = np.asarray(inputs["v_in"], dtype=np.float32)
    rel_pos_bias = np.asarray(inputs["rel_pos_bias"], dtype=np.float32)
    Wq = np.asarray(inputs["Wq"], dtype=np.float32)
    Wk = np.asarray(inputs["Wk"], dtype=np.float32)
    Wv = np.asarray(inputs["Wv"], dtype=np.float32)
    Wp = np.asarray(inputs["Wp"], dtype=np.float32)
    bp = np.asarray(inputs["bp"], dtype=np.float32)

    nc = get_program()
    in_maps = make_in_maps(x, k_in, v_in, rel_pos_bias, Wq, Wk, Wv, Wp)
    res = run_bass_kernel_spmd(nc, in_maps, list(range(8)))
    return assemble_output(res.results, bp)
